# revision 8
# baseline (speedup 1.0000x reference)
"""LSNN cell single-step kernel for Trainium2, data-parallel over 8 NeuronCores.

Full-input contract: kernel(**inputs) takes the unsharded tensors
(B=8192, IN_F=512, OUT_F=1024) and returns the stacked [4, B, OUT_F]
(z_new, v_new, i_new, b_new) fp32 output.

Sharding: batch 8192 -> 8 cores x 1024 rows. Weights are replicated,
host-transposed to [K, N] ("rhs") layout and cast to bf16 (the spike
matmul operands are exactly 0/1 so the only rounding is in the weights).
All threshold-feeding elementwise math is fp32 with the reference's
exact operation order so z_new/v_new/b_new are bit-exact.

Per-core structure: batch tiles of 128 rows. DMA rings are separated by
role so loads never queue behind compute-gated stores (HWDGE rings are
FIFO per issuing engine): all input loads stream on SyncE's ring, the
weights stream in parallel on ScalarE's ring at the start, and the four
output stores go through GpSimdE SWDGE. i_dec is injected into the
matmul PSUM accumulation via an identity matmul so i_new exits PSUM
through a single ScalarE copy.
"""

import sys
import types
from contextlib import ExitStack

import numpy as np
import ml_dtypes

# bass_utils imports antenv.axon_hooks when tracing is requested (e.g. via a
# BASS_TRACE env var); this image's antenv package lacks that module. Register
# a fallback shim that reports "no hook" so tracing degrades instead of
# crashing. test.py overwrites the getter with a real ctypes-backed hook.
if "antenv.axon_hooks" not in sys.modules:
    _shim = types.ModuleType("antenv.axon_hooks")
    _shim._hook = None
    _shim.get_axon_ntff_profile_hook = lambda: _shim._hook

    def _set_hook(h):
        _shim._hook = h

    _shim.set_axon_ntff_profile_hook = _set_hook
    import antenv  # noqa: F401  (make the parent package importable first)

    sys.modules["antenv.axon_hooks"] = _shim

import concourse.bass as bass
import concourse.tile as tile
from concourse import bacc, mybir
from concourse.bass_utils import run_bass_kernel_spmd
from concourse.masks import make_identity

F32 = mybir.dt.float32
BF16 = mybir.dt.bfloat16
ALU = mybir.AluOpType
ACT_COPY = mybir.ActivationFunctionType.Copy

N_CORES = 8
B, IN_F, OUT_F = 8192, 512, 1024
B_CORE = B // N_CORES          # 1024 rows per core
P = 128                        # partitions
KI = IN_F // P                 # 4 contraction chunks for the input matmul
KO = OUT_F // P                # 8 contraction chunks for the recurrent matmul
NH = OUT_F // 2                # 512-wide PSUM half (one bank)

# Constants, replicating the reference's jax fp32 arithmetic exactly.
# python-double products are cast to fp32 once multiplied with fp32 arrays;
# instruction immediates are stored as fp32, giving the same cast.
C_VDEC = 0.001 * 100.0                   # DT * TAU_MEM_INV
C_BDEC = 0.001 * (1.0 / 800.0)           # DT * TAU_ADAPT_INV
C_IDEC = 0.001 * (-200.0)                # DT * (-TAU_SYN_INV)
# reference computes (z * f32(TAU_ADAPT_INV)) * f32(BETA); with z in {0,1}
# that's z * (f32(1/800) *f32 f32(1.8)) exactly.
C_BJUMP = float(np.float32(np.float32(1.0 / 800.0) * np.float32(1.8)))


def build_nc(n_btiles: int = B_CORE // P):
    """Emit the per-core Tile kernel for `n_btiles` batch tiles of 128."""
    rows = n_btiles * P
    nc = bacc.Bacc(
        "TRN2",
        target_bir_lowering=False,
        debug=False,
        enable_asserts=False,
        num_devices=N_CORES,
    )
    s_d = nc.dram_tensor("in_spikes", [rows, IN_F], F32, kind="ExternalInput").ap()
    z_d = nc.dram_tensor("in_z", [rows, OUT_F], F32, kind="ExternalInput").ap()
    v_d = nc.dram_tensor("in_v", [rows, OUT_F], F32, kind="ExternalInput").ap()
    i_d = nc.dram_tensor("in_i", [rows, OUT_F], F32, kind="ExternalInput").ap()
    b_d = nc.dram_tensor("in_b", [rows, OUT_F], F32, kind="ExternalInput").ap()
    wiT_d = nc.dram_tensor("in_wiT", [IN_F, OUT_F], BF16, kind="ExternalInput").ap()
    wrT_d = nc.dram_tensor("in_wrT", [OUT_F, OUT_F], BF16, kind="ExternalInput").ap()
    out_d = nc.dram_tensor("out", [4, rows, OUT_F], F32, kind="ExternalOutput").ap()

    with tile.TileContext(nc) as tc, ExitStack() as ctx:
        const_pool = ctx.enter_context(tc.tile_pool(name="const", bufs=1))
        w_pool = ctx.enter_context(tc.tile_pool(name="weights", bufs=1))
        in_pool = ctx.enter_context(tc.tile_pool(name="inp", bufs=4))
        lhsT_pool = ctx.enter_context(tc.tile_pool(name="lhsT", bufs=2))
        tmp_pool = ctx.enter_context(tc.tile_pool(name="tmp", bufs=2))
        vdec_pool = ctx.enter_context(tc.tile_pool(name="vdec", bufs=3))
        out_pool = ctx.enter_context(tc.tile_pool(name="outp", bufs=3))
        psum_tr = ctx.enter_context(
            tc.tile_pool(name="psum_tr", bufs=2, space="PSUM")
        )
        psum_mm = ctx.enter_context(
            tc.tile_pool(name="psum_mm", bufs=2, space="PSUM")
        )

        ident = const_pool.tile([P, P], F32)
        make_identity(nc, ident)
        ident_bf = const_pool.tile([P, P], BF16)
        make_identity(nc, ident_bf)

        # Weights stream on ScalarE's HWDGE ring, in parallel with the input
        # stream on SyncE's ring. wrT first (the z matmuls dominate), halved
        # so the first chunks land early.
        wrT = w_pool.tile([P, KO, OUT_F], BF16)
        wrT_v = wrT_d.rearrange("(c p) n -> p c n", p=P)
        nc.scalar.dma_start(wrT[:, : KO // 2, :], wrT_v[:, : KO // 2, :])
        nc.scalar.dma_start(wrT[:, KO // 2 :, :], wrT_v[:, KO // 2 :, :])
        wiT = w_pool.tile([P, KI, OUT_F], BF16)
        nc.scalar.dma_start(wiT, wiT_d.rearrange("(c p) n -> p c n", p=P))

        for t in range(n_btiles):
            rs = bass.ts(t, P)  # this tile's 128 batch rows in DRAM

            z_t = in_pool.tile([P, OUT_F], F32, tag="z")
            nc.sync.dma_start(z_t, z_d[rs, :])
            i_t = in_pool.tile([P, OUT_F], F32, tag="i")
            nc.sync.dma_start(i_t, i_d[rs, :])
            v_t = in_pool.tile([P, OUT_F], F32, tag="v")
            nc.sync.dma_start(v_t, v_d[rs, :])
            b_t = in_pool.tile([P, OUT_F], F32, tag="b")
            nc.sync.dma_start(b_t, b_d[rs, :])
            s_t = in_pool.tile([P, IN_F], F32, tag="s")
            nc.sync.dma_start(s_t, s_d[rs, :])

            # Head of the elementwise chain first, so each engine's stream
            # opens with work that is ready as soon as the loads land (the
            # per-engine instruction streams are FIFO; putting the transpose
            # copies ahead of these would stall the DVE chain behind them).
            idec = tmp_pool.tile([P, OUT_F], BF16, tag="idec")
            nc.scalar.activation(idec, i_t, ACT_COPY, scale=0.8)
            bdec = tmp_pool.tile([P, OUT_F], F32, tag="bdec")
            nc.scalar.activation(bdec, b_t, ACT_COPY, bias=1.0, scale=-1.0)
            vdec = vdec_pool.tile([P, OUT_F], F32, tag="vdec")
            nc.vector.tensor_tensor(vdec, i_t, v_t, ALU.subtract)
            nc.vector.scalar_tensor_tensor(
                vdec, vdec, C_VDEC, v_t, ALU.mult, ALU.add
            )
            nc.vector.scalar_tensor_tensor(
                bdec, bdec, C_BDEC, b_t, ALU.mult, ALU.add
            )
            nz = vdec_pool.tile([P, OUT_F], F32, tag="nz")
            nc.vector.tensor_tensor(nz, vdec, bdec, ALU.is_le)  # 1 - z_new

            # Transpose the spike operands 128x128-blockwise on PE (paired
            # into 2-bank PSUM tiles) and cast to bf16 on the way out; these
            # become matmul lhsT.
            zT = lhsT_pool.tile([P, KO, P], BF16, tag="zT")
            for k in range(0, KO, 2):
                ps = psum_tr.tile([P, 2, P], F32, tag="tr")
                nc.tensor.transpose(ps[:, 0, :], z_t[:, bass.ts(k, P)], ident)
                nc.tensor.transpose(ps[:, 1, :], z_t[:, bass.ts(k + 1, P)], ident)
                nc.scalar.activation(zT[:, k : k + 2, :], ps, ACT_COPY)
            sT = lhsT_pool.tile([P, KI, P], BF16, tag="sT")
            for k in range(0, KI, 2):
                ps = psum_tr.tile([P, 2, P], F32, tag="tr")
                nc.tensor.transpose(ps[:, 0, :], s_t[:, bass.ts(k, P)], ident)
                nc.tensor.transpose(ps[:, 1, :], s_t[:, bass.ts(k + 1, P)], ident)
                nc.scalar.activation(sT[:, k : k + 2, :], ps, ACT_COPY)

            # acc[:, j, :] = z @ WrT + spikes @ WiT + i_dec
            acc = psum_mm.tile([P, 2, NH], F32, tag="mm")
            for j in range(2):
                ns = bass.ts(j, NH)
                for k in range(KO):
                    nc.tensor.matmul(
                        acc[:, j, :], zT[:, k, :], wrT[:, k, ns],
                        start=(k == 0), stop=False,
                    )
                for k in range(KI):
                    nc.tensor.matmul(
                        acc[:, j, :], sT[:, k, :], wiT[:, k, ns],
                        start=False, stop=False,
                    )
                nc.tensor.matmul(
                    acc[:, j, :], ident_bf, idec[:, ns],
                    start=False, stop=True,
                )

            # Tail of the elementwise chain.
            v_o = out_pool.tile([P, OUT_F], F32, tag="vo")
            nc.vector.tensor_tensor(v_o, vdec, nz, ALU.mult)
            z_o = out_pool.tile([P, OUT_F], F32, tag="zo")
            nc.scalar.activation(z_o, nz, ACT_COPY, bias=1.0, scale=-1.0)

            i_o = out_pool.tile([P, OUT_F], F32, tag="io")
            nc.scalar.activation(
                i_o.rearrange("p (a n) -> p a n", a=2), acc, ACT_COPY
            )

            b_o = out_pool.tile([P, OUT_F], F32, tag="bo")
            nc.vector.scalar_tensor_tensor(
                b_o, z_o, C_BJUMP, bdec, ALU.mult, ALU.add
            )

            nc.gpsimd.dma_start(out_d[0, rs, :], z_o)
            nc.gpsimd.dma_start(out_d[1, rs, :], v_o)
            nc.gpsimd.dma_start(out_d[2, rs, :], i_o)
            nc.gpsimd.dma_start(out_d[3, rs, :], b_o)

    nc.compile()
    return nc


_NC_CACHE = {}


def _get_nc(n_btiles: int = B_CORE // P):
    if n_btiles not in _NC_CACHE:
        _NC_CACHE[n_btiles] = build_nc(n_btiles)
    return _NC_CACHE[n_btiles]


def make_in_maps(input_spikes, z, v, i, b, input_weights, recurrent_weights):
    """Shard full inputs into per-core in_maps (batch split, weights repl)."""
    wiT = np.ascontiguousarray(
        np.asarray(input_weights, dtype=np.float32).T
    ).astype(ml_dtypes.bfloat16)
    wrT = np.ascontiguousarray(
        np.asarray(recurrent_weights, dtype=np.float32).T
    ).astype(ml_dtypes.bfloat16)
    maps = []
    for c in range(N_CORES):
        sl = slice(c * B_CORE, (c + 1) * B_CORE)
        maps.append(
            {
                "in_spikes": np.ascontiguousarray(input_spikes[sl], np.float32),
                "in_z": np.ascontiguousarray(z[sl], np.float32),
                "in_v": np.ascontiguousarray(v[sl], np.float32),
                "in_i": np.ascontiguousarray(i[sl], np.float32),
                "in_b": np.ascontiguousarray(b[sl], np.float32),
                "in_wiT": wiT,
                "in_wrT": wrT,
            }
        )
    return maps


def run_sharded(inputs: dict, trace: bool = False, **kw):
    """Compile (cached), run on 8 cores, return (full_output, raw_results)."""
    nc = _get_nc()
    in_maps = make_in_maps(**inputs)
    res = run_bass_kernel_spmd(
        nc, in_maps, list(range(N_CORES)), trace=trace, **kw
    )
    out = np.empty((4, B, OUT_F), dtype=np.float32)
    for c in range(N_CORES):
        out[:, c * B_CORE : (c + 1) * B_CORE, :] = res.results[c]["out"]
    return out, res


def kernel(**inputs) -> np.ndarray:
    out, _ = run_sharded(inputs, trace=False)
    return out
